# revision 2
# baseline (speedup 1.0000x reference)
"""Episodic-memory retrieval kernel for 8 Trainium2 NeuronCores.

Pipeline (classic sharded ANN retrieval, capacity-axis sharding):
  K1: per-core cosine sim (queries x embedding-shard) + local top-8
  host: merge candidates -> global top-8, build gather/scatter indices
  K2: per-core indirect-DMA gather of owned winners from episode shards,
      scatter into a compacted per-core output; host reassembles.
"""

import numpy as np

import concourse.bass as bass
import concourse.bacc as bacc
import concourse.mybir as mybir
from concourse.bass import IndirectOffsetOnAxis
from concourse.bass_utils import run_bass_kernel_spmd
from concourse.masks import make_identity
from concourse.tile import TileContext

F32 = mybir.dt.float32
I32 = mybir.dt.int32
U32 = mybir.dt.uint32

B, H, C, S, CS, K = 256, 256, 16384, 64, 32, 8
NCORES = 8
CSH = C // NCORES        # 2048 episodes per core shard
EPS = 1e-8
NB = 4                   # K2 batches of 128 winner-slots -> capacity 512/core
CAP = NB * 128
ROW = S * H              # 16384 f32 per full episode
CROW = CS * H            # 8192 f32 per compressed episode

_progs = {}


def _build_k1():
    nc = bacc.Bacc(None, target_bir_lowering=False)
    q = nc.dram_tensor("q", [B, H], F32, kind="ExternalInput")
    e = nc.dram_tensor("e", [CSH, H], F32, kind="ExternalInput")
    scores = nc.dram_tensor("scores", [B, K], F32, kind="ExternalOutput")
    idx = nc.dram_tensor("idx", [B, K], U32, kind="ExternalOutput")

    with TileContext(nc) as tc:
        with (
            tc.tile_pool(name="work", bufs=3) as wp,
            tc.tile_pool(name="psum", bufs=4, space="PSUM") as pp,
            tc.tile_pool(name="persist", bufs=1) as keep,
        ):
            ident = keep.tile([128, 128], F32, tag="ident")
            make_identity(nc, ident[:])

            def normalize(t):
                # t: [128, H] rows; t <- t / max(||t||, EPS) per row
                sq = wp.tile([128, H], F32, tag="sq")
                ssq = wp.tile([128, 1], F32, tag="ssq")
                nc.scalar.activation(
                    out=sq[:], in_=t[:],
                    func=mybir.ActivationFunctionType.Square,
                    accum_out=ssq[:],
                )
                nrm = wp.tile([128, 1], F32, tag="nrm")
                nc.scalar.activation(
                    out=nrm[:], in_=ssq[:],
                    func=mybir.ActivationFunctionType.Sqrt,
                )
                nc.vector.tensor_scalar_max(nrm[:], nrm[:], EPS)
                rn = wp.tile([128, 1], F32, tag="rn")
                nc.vector.reciprocal(rn[:], nrm[:])
                nc.vector.tensor_scalar_mul(t[:], t[:], rn[:])

            # qT[kt] : [128 (h), B] with h-slice kt
            qT = [keep.tile([128, B], F32, tag=f"qT{i}", name=f"qT{i}") for i in range(H // 128)]
            for mt in range(B // 128):
                qt = wp.tile([128, H], F32, tag="qt")
                nc.sync.dma_start(out=qt[:], in_=q[mt * 128:(mt + 1) * 128, :])
                normalize(qt)
                for kt in range(H // 128):
                    pt = pp.tile([128, 128], F32, space="PSUM", tag="pt")
                    nc.tensor.transpose(
                        out=pt[:], in_=qt[:, kt * 128:(kt + 1) * 128],
                        identity=ident[:],
                    )
                    nc.vector.tensor_copy(
                        out=qT[kt][:, mt * 128:(mt + 1) * 128], in_=pt[:]
                    )

            # eT[kt] : [128 (h), CSH]
            eT = [keep.tile([128, CSH], F32, tag=f"eT{i}", name=f"eT{i}") for i in range(H // 128)]
            for ct in range(CSH // 128):
                et = wp.tile([128, H], F32, tag="et")
                nc.sync.dma_start(out=et[:], in_=e[ct * 128:(ct + 1) * 128, :])
                normalize(et)
                for kt in range(H // 128):
                    pt = pp.tile([128, 128], F32, space="PSUM", tag="pt")
                    nc.tensor.transpose(
                        out=pt[:], in_=et[:, kt * 128:(kt + 1) * 128],
                        identity=ident[:],
                    )
                    nc.vector.tensor_copy(
                        out=eT[kt][:, ct * 128:(ct + 1) * 128], in_=pt[:]
                    )

            # sim[mt] : [128 (query), CSH], then top-8 per row
            for mt in range(B // 128):
                simt = keep.tile([128, CSH], F32, tag=f"sim{mt}", name=f"sim{mt}")
                for nt in range(CSH // 512):
                    ps = pp.tile([128, 512], F32, space="PSUM", tag="ps")
                    for kt in range(H // 128):
                        nc.tensor.matmul(
                            out=ps[:],
                            lhsT=qT[kt][:, mt * 128:(mt + 1) * 128],
                            rhs=eT[kt][:, nt * 512:(nt + 1) * 512],
                            start=(kt == 0),
                            stop=(kt == H // 128 - 1),
                        )
                    nc.vector.tensor_copy(
                        out=simt[:, nt * 512:(nt + 1) * 512], in_=ps[:]
                    )
                s8 = wp.tile([128, 8], F32, tag="s8")
                nc.vector.max(out=s8[:], in_=simt[:])
                i8 = wp.tile([128, 8], U32, tag="i8")
                nc.vector.max_index(out=i8[:], in_max=s8[:], in_values=simt[:])
                nc.sync.dma_start(
                    out=scores[mt * 128:(mt + 1) * 128, :], in_=s8[:]
                )
                nc.sync.dma_start(out=idx[mt * 128:(mt + 1) * 128, :], in_=i8[:])

    nc.compile()
    return nc


def _build_k2():
    nc = bacc.Bacc(None, target_bir_lowering=False)
    ep = nc.dram_tensor("ep", [CSH, ROW], F32, kind="ExternalInput")
    cp = nc.dram_tensor("cp", [CSH, CROW], F32, kind="ExternalInput")
    gidx = nc.dram_tensor("gidx", [CAP, 1], I32, kind="ExternalInput")
    cidx = nc.dram_tensor("cidx", [CAP, 1], I32, kind="ExternalInput")
    slot0 = nc.dram_tensor("slot0", [CAP, 1], I32, kind="ExternalInput")
    slot1 = nc.dram_tensor("slot1", [CAP, 1], I32, kind="ExternalInput")
    out = nc.dram_tensor("out", [CAP, ROW], F32, kind="ExternalOutput")
    out_half = out[:].rearrange("w (h x) -> (w h) x", h=2)  # [2*CAP, CROW]

    with TileContext(nc) as tc:
        with (
            tc.tile_pool(name="data", bufs=2) as dp,
            tc.tile_pool(name="idxp", bufs=2 * NB) as ip,
        ):
            for bt in range(NB):
                sl = slice(bt * 128, (bt + 1) * 128)
                gi = ip.tile([128, 1], I32, tag="gi")
                ci = ip.tile([128, 1], I32, tag="ci")
                s0 = ip.tile([128, 1], I32, tag="s0")
                s1 = ip.tile([128, 1], I32, tag="s1")
                nc.sync.dma_start(out=gi[:], in_=gidx[sl, :])
                nc.sync.dma_start(out=ci[:], in_=cidx[sl, :])
                nc.sync.dma_start(out=s0[:], in_=slot0[sl, :])
                nc.sync.dma_start(out=s1[:], in_=slot1[sl, :])

                t = dp.tile([128, ROW], F32, tag="t")
                # full episodes for owned non-compressed winners
                nc.gpsimd.indirect_dma_start(
                    out=t[:],
                    out_offset=None,
                    in_=ep[:],
                    in_offset=IndirectOffsetOnAxis(ap=gi[:, :1], axis=0),
                    bounds_check=CSH - 1,
                    oob_is_err=False,
                )
                # compressed episodes for owned compressed winners (first half)
                nc.gpsimd.indirect_dma_start(
                    out=t[:, :CROW],
                    out_offset=None,
                    in_=cp[:],
                    in_offset=IndirectOffsetOnAxis(ap=ci[:, :1], axis=0),
                    bounds_check=CSH - 1,
                    oob_is_err=False,
                )
                # first half-row out (valid for every owned winner)
                nc.gpsimd.indirect_dma_start(
                    out=out_half,
                    out_offset=IndirectOffsetOnAxis(ap=s0[:, :1], axis=0),
                    in_=t[:, :CROW],
                    in_offset=None,
                    bounds_check=2 * CAP - 1,
                    oob_is_err=False,
                )
                # second half-row out (non-compressed only; compressed stays
                # zero from the pre-zeroed output buffer)
                nc.gpsimd.indirect_dma_start(
                    out=out_half,
                    out_offset=IndirectOffsetOnAxis(ap=s1[:, :1], axis=0),
                    in_=t[:, CROW:],
                    in_offset=None,
                    bounds_check=2 * CAP - 1,
                    oob_is_err=False,
                )

    nc.compile()
    return nc


def _get(name):
    if name not in _progs:
        _progs[name] = _build_k1() if name == "k1" else _build_k2()
    return _progs[name]


def _run_k1(query, emb, trace=False):
    nc = _get("k1")
    q = np.ascontiguousarray(query, dtype=np.float32)
    in_maps = [
        {"q": q, "e": np.ascontiguousarray(emb[c * CSH:(c + 1) * CSH])}
        for c in range(NCORES)
    ]
    return run_bass_kernel_spmd(
        nc, in_maps, core_ids=list(range(NCORES)), trace=trace
    )


def _run_k2(in_maps, trace=False):
    nc = _get("k2")
    return run_bass_kernel_spmd(
        nc, in_maps, core_ids=list(range(NCORES)), trace=trace
    )


def kernel(query, episode_embeddings, episodes, compressed_episodes,
           is_compressed, k, _trace=False, _results=None):
    assert int(k) == K
    r1 = _run_k1(query, episode_embeddings, trace=_trace)

    # host: merge the 8 per-shard candidate lists -> global top-8
    cand_s = np.concatenate(
        [r1.results[c]["scores"] for c in range(NCORES)], axis=1
    )  # [B, 64]
    cand_i = np.concatenate(
        [r1.results[c]["idx"].astype(np.int64) + c * CSH for c in range(NCORES)],
        axis=1,
    )
    order = np.argsort(-cand_s, axis=1, kind="stable")[:, :K]
    top_scores = np.take_along_axis(cand_s, order, axis=1)
    top_idx = np.take_along_axis(cand_i, order, axis=1)  # [B, K] global

    # host: per-core gather/scatter index tensors
    comp = np.asarray(is_compressed).astype(bool)
    flat_idx = top_idx.reshape(-1)              # [B*K] winner slot w -> episode
    flat_comp = comp[flat_idx]
    owner = flat_idx // CSH
    ep_flat = np.ascontiguousarray(
        np.asarray(episodes, dtype=np.float32).reshape(C, ROW)
    )
    cp_flat = np.ascontiguousarray(
        np.asarray(compressed_episodes, dtype=np.float32).reshape(C, CROW)
    )

    in2 = []
    owned_w = []
    for c in range(NCORES):
        w = np.nonzero(owner == c)[0]           # winner slots owned by core c
        n = len(w)
        assert n <= CAP, f"core {c} owns {n} winners > capacity {CAP}"
        owned_w.append(w)
        li = (flat_idx[w] - c * CSH).astype(np.int32)   # local table rows
        fc = flat_comp[w]
        gidx = np.full(CAP, CSH, np.int32)
        cidx = np.full(CAP, CSH, np.int32)
        s0 = np.full(CAP, 2 * CAP, np.int32)
        s1 = np.full(CAP, 2 * CAP, np.int32)
        gidx[:n] = np.where(fc, CSH, li)
        cidx[:n] = np.where(fc, li, CSH)
        r = np.arange(n)
        s0[:n] = 2 * r
        s1[:n] = np.where(fc, 2 * CAP, 2 * r + 1)
        in2.append({
            "ep": ep_flat[c * CSH:(c + 1) * CSH],
            "cp": cp_flat[c * CSH:(c + 1) * CSH],
            "gidx": gidx.reshape(CAP, 1),
            "cidx": cidx.reshape(CAP, 1),
            "slot0": s0.reshape(CAP, 1),
            "slot1": s1.reshape(CAP, 1),
        })

    r2 = _run_k2(in2, trace=_trace)

    retrieved = np.empty((B * K, S, H), dtype=np.float32)
    for c in range(NCORES):
        w = owned_w[c]
        retrieved[w] = r2.results[c]["out"][:len(w)].reshape(-1, S, H)
    retrieved = retrieved.reshape(B, K, S, H)

    if _results is not None:
        _results["r1"] = r1
        _results["r2"] = r2
    return retrieved, top_scores.astype(np.float32)
